# revision 27
# baseline (speedup 1.0000x reference)
"""Distributed KNN retrieval (Database topk=4) on 8 Trainium2 NeuronCores.

Pipeline (per core, SPMD over 8 cores; corpus sharded along N):
  1. Phase-1 scan of the core's 50000-column shard in 2048-column chunks:
     fp8e4 DoubleRow matmul (2 MACs/cell/cycle; raw queries -- per-query
     ranking is scale invariant) -> PSUM fp32 sims.
  2. ScalarE packs value+index in ONE pass: it copies PSUM fp32 -> bf16
     with a stride-2 write into the HIGH halfwords of a u32 tile whose low
     halfwords are pre-filled with the column iota. The u32 tile read as
     fp32 orders by similarity (low 16 bits are sub-ulp noise) and its low
     bits carry the in-chunk column.
  3. DVE max8 per chunk on packed -> per-core candidate list [128, 200].
     Level-2: max8 + find_index8 over the candidates; the candidate position
     gives the chunk (p>>3), the packed low bits give the column. No
     indirect-DMA hop is needed to resolve indices.
  4. Exact fp32 rescore: per-candidate indirect-DMA gathers of the 6 winner
     rows from the fp32 shard, then fused mult+accum on DVE.
Host merges 8 cores x 6 exact-scored candidates -> global top-4.

The masked range [start, end) is zeroed in the fp8 shard: masked sims are
exactly 0 and never reach the per-core top-8 (top sims are strictly
positive); the fp32 rescore table keeps original values so outputs stay
exact.
"""

import os

import numpy as np
import ml_dtypes

import concourse.bass as bass
import concourse.bacc as bacc
import concourse.mybir as mybir
import concourse.tile as tile
import concourse.bass_utils as bass_utils

Q, D, N, TOPK = 256, 768, 400000, 4
NCORES = 8
NSHARD = N // NCORES          # 50000
CHUNK = 2048
CHUNK_SH = 11                 # log2(CHUNK)
NCH = (NSHARD + CHUNK - 1) // CHUNK   # 25
NPAD = NCH * CHUNK            # 51200
KT2 = D // 256                # 3 DoubleRow k-tiles (256-deep each)
MT = Q // 128                 # 2 m-tiles
CAND = NCH * 8                # 200 level-1 candidates per core per query
L2K = 6                       # candidates rescored per core per query
NPK = 4                       # packed-tile ring depth
LASTPAD = 1024                # padded width scanned in the last chunk
EMB_SCALE = 512.0             # fp8 range scaling (ranking invariant)

_prog_cache = {}


def _install_ntff_hook_shim():
    """Provide antenv.axon_hooks (absent in this image) so that
    run_bass_kernel_spmd(trace=True) can capture NTFF profiles through the
    injected libaxon_pjrt.so. Mirrors trn_agent_boot/trn_boot.py."""
    import sys
    import types
    import ctypes
    import contextlib

    if "antenv.axon_hooks" in sys.modules:
        return
    mod = types.ModuleType("antenv.axon_hooks")
    state = {"hook": None}
    mod.set_axon_ntff_profile_hook = lambda h: state.__setitem__("hook", h)
    mod.get_axon_ntff_profile_hook = lambda: state["hook"]
    sys.modules["antenv.axon_hooks"] = mod

    so_path = "/opt/axon/libaxon_pjrt.so"
    if not os.path.exists(so_path):
        return
    try:
        lib = ctypes.CDLL(so_path)
    except OSError:
        return
    if not hasattr(lib, "axon_start_nrt_profile"):
        return
    lib.axon_start_nrt_profile.argtypes = [ctypes.POINTER(ctypes.c_int64),
                                           ctypes.c_size_t]
    lib.axon_start_nrt_profile.restype = ctypes.c_int64
    lib.axon_stop_nrt_profile.argtypes = [ctypes.c_char_p]
    lib.axon_stop_nrt_profile.restype = ctypes.c_int64

    @contextlib.contextmanager
    def _hook(output_dir, device_ids):
        import jax
        jax.devices()
        if device_ids:
            ids = (ctypes.c_int64 * len(device_ids))(*device_ids)
            rc = lib.axon_start_nrt_profile(ids, len(device_ids))
        else:
            rc = lib.axon_start_nrt_profile(None, 0)
        if rc != 0:
            raise RuntimeError(f"axon_start_nrt_profile rc={rc}")
        try:
            yield
        finally:
            n = lib.axon_stop_nrt_profile(str(output_dir).encode())
            print(f"ntff profile: {n} file(s) written to {output_dir}")

    mod.set_axon_ntff_profile_hook(_hook)


def _build_program():
    nc = bacc.Bacc(None, target_bir_lowering=False, debug=False)
    f8 = mybir.dt.float8e4
    u32 = mybir.dt.uint32
    f32 = mybir.dt.float32
    bf16 = mybir.dt.bfloat16

    q_dram = nc.dram_tensor("q", [Q, D], f32, kind="ExternalInput")
    # raw queries, fp8, pre-transposed on host: qt[t, i, p, m] = q8[m, t*256+i*128+p]
    qt_dram = nc.dram_tensor("qT", [KT2, 2, 128, Q], f8, kind="ExternalInput")
    # emb shard, fp8, host-packed DoubleRow layout:
    # embL[j, p, (t*2 + i)*CHUNK + n] = emb8[t*256 + i*128 + p, j*CHUNK + n]
    embL = nc.dram_tensor("embL", [NCH, 128, KT2 * 2 * CHUNK], f8,
                          kind="ExternalInput")
    # fp32 shard transposed (rows = corpus columns) for the exact rescore
    embT = nc.dram_tensor("embT", [NSHARD, D], f32, kind="ExternalInput")

    out_vals = nc.dram_tensor("out_vals", [Q, L2K], f32, kind="ExternalOutput")
    out_ids = nc.dram_tensor("out_ids", [Q, L2K], u32, kind="ExternalOutput")

    AND = mybir.AluOpType.bitwise_and
    SHR = mybir.AluOpType.logical_shift_right
    SHL = mybir.AluOpType.logical_shift_left
    ADD = mybir.AluOpType.add
    MUL = mybir.AluOpType.mult
    BYP = mybir.AluOpType.bypass
    DR = mybir.MatmulPerfMode.DoubleRow

    with tile.TileContext(nc) as tc:
        with tc.tile_pool(name="persist", bufs=1) as pp:
            qn = [pp.tile([128, D], f32, tag=f"qn{m}", name=f"qn{m}")
                  for m in range(MT)]
            qT = pp.tile([128, KT2, 2, Q], f8, tag="qT")
            vals_all = [pp.tile([128, CAND], f32, tag=f"va{m}", name=f"va{m}")
                        for m in range(MT)]
            # packed ring: low halfwords hold the column iota permanently
            pk = [pp.tile([128, CHUNK], u32, tag=f"pk{i}", name=f"pk{i}")
                  for i in range(NPK)]

            nc.sync.dma_start(qT[:], qt_dram.ap().rearrange("t i p q -> p t i q"))
            for i in range(NPK):
                nc.gpsimd.iota(pk[i][:], pattern=[[1, CHUNK]], base=0,
                               channel_multiplier=0)

            cand = [pp.tile([128, L2K, D], f32, tag=f"cand{m}",
                            name=f"cand{m}") for m in range(MT)]
            resc = [pp.tile([128, L2K], f32, tag=f"resc{m}", name=f"resc{m}")
                    for m in range(MT)]
            scratch = [pp.tile([128, D], f32, tag=f"scr{m}", name=f"scr{m}")
                       for m in range(MT)]

            def level2_and_gather(m):
                """max8+find over vals_all[m], decode global shard column,
                emit out_ids and the per-candidate row gathers."""
                top = pp.tile([128, 8], f32, tag=f"top{m}")
                p8 = pp.tile([128, 8], u32, tag=f"p8{m}")
                cb = pp.tile([128, 8], u32, tag=f"cb{m}")
                gid = pp.tile([128, 8], u32, tag=f"gid{m}")
                gc = pp.tile([128, 8], u32, tag=f"gc{m}")
                nc.vector.max(top[:], vals_all[m][:])
                nc.vector.max_index(p8[:], top[:], vals_all[m][:])
                # shard col = (p>>3)<<CHUNK_SH | (packed & (CHUNK-1))
                nc.vector.tensor_scalar(cb[:], p8[:], 3, CHUNK_SH,
                                        op0=SHR, op1=SHL)
                nc.vector.tensor_scalar(gid[:], top[:].bitcast(u32),
                                        CHUNK - 1, None, op0=AND)
                nc.vector.tensor_tensor(gid[:], gid[:], cb[:], op=ADD)
                nc.sync.dma_start(
                    out_ids.ap()[m * 128:(m + 1) * 128, :], gid[:, :L2K])
                nc.vector.tensor_scalar_min(gc[:], gid[:], float(NSHARD - 1))
                # per-candidate [128,1]-offset DMAs: the only indirect
                # pattern that works on real HW (batched [128,L2K] offsets
                # pass CoreSim but return garbage on device)
                for r in range(L2K):
                    nc.gpsimd.indirect_dma_start(
                        out=cand[m][:, r, :], out_offset=None,
                        in_=embT.ap()[:],
                        in_offset=bass.IndirectOffsetOnAxis(
                            ap=gc[:, r:r + 1], axis=0))

            def rescore(m):
                for r in range(L2K):
                    nc.vector.scalar_tensor_tensor(
                        scratch[m][:], cand[m][:, r, :], 1.0, qn[m][:],
                        op0=BYP, op1=MUL, accum_out=resc[m][:, r:r + 1])
                nc.sync.dma_start(
                    out_vals.ap()[m * 128:(m + 1) * 128, :], resc[m][:])

            # ---------- phase 1: scan shard ----------
            with (
                tc.tile_pool(name="rhs_sb", bufs=4) as rp,
                tc.tile_pool(name="sim_ps", bufs=2, space="PSUM") as sps,
            ):
                for j in range(NCH):
                    w = LASTPAD if j == NCH - 1 else CHUNK
                    rhs = rp.tile([128, KT2, 2, CHUNK], f8, tag="rhs")
                    nc.sync.dma_start(
                        rhs[:, :, :, :w],
                        embL.ap()[j].rearrange("p (t i n) -> p t i n",
                                               t=KT2, i=2)[:, :, :, :w])
                    for m in range(MT):
                        psum = sps.tile([128, CHUNK], f32, tag="sim")
                        for t in range(KT2):
                            for h in range(w // 512):
                                nc.tensor.matmul(
                                    psum[:, h * 512:(h + 1) * 512],
                                    qT[:, t, :, m * 128:(m + 1) * 128],
                                    rhs[:, t, :, h * 512:(h + 1) * 512],
                                    start=(t == 0), stop=(t == KT2 - 1),
                                    perf_mode=DR)
                        packed = pk[(j * MT + m) % NPK]
                        nc.scalar.copy(
                            packed[:].bitcast(bf16)[:, 1:2 * w:2],
                            psum[:, :w])
                        nc.vector.max(vals_all[m][:, j * 8:(j + 1) * 8],
                                      packed[:, :w].bitcast(f32))
                        # fire m's level-2 as soon as its last chunk lands
                        if j == NCH - 1:
                            level2_and_gather(m)

            # ---------- query normalize (feeds the exact rescore only) -------
            # issued after the scan so its DMAs/ops don't delay the first
            # rhs-chunk load; it still completes well before the rescore
            with tc.tile_pool(name="prep_sb", bufs=2) as sp:
                for m in range(MT):
                    q_sb = sp.tile([128, D], f32, tag="qsb")
                    nc.sync.dma_start(q_sb[:],
                                      q_dram.ap()[m * 128:(m + 1) * 128, :])
                    ssum = sp.tile([128, 1], f32, tag="ssum")
                    nc.vector.tensor_reduce(ssum[:], q_sb[:],
                                            axis=mybir.AxisListType.X,
                                            op=mybir.AluOpType.add,
                                            apply_absolute_value=True)
                    nc.vector.tensor_scalar_max(ssum[:], ssum[:], 1e-12)
                    rcp = sp.tile([128, 1], f32, tag="rcp")
                    nc.vector.reciprocal(rcp[:], ssum[:])
                    nc.scalar.mul(qn[m][:], q_sb[:], rcp[:])

            # ---------- exact rescore (tail) ----------
            for m in range(MT):
                rescore(m)

    nc.compile()
    return nc


def _get_program():
    if "nc" not in _prog_cache:
        _prog_cache["nc"] = _build_program()
    return _prog_cache["nc"]


def _prepare_core_inputs(q, emb, start, end):
    """Shard + pack inputs for each core. Returns list of per-core dicts."""
    f8 = ml_dtypes.float8_e4m3
    embs = emb * np.float32(EMB_SCALE)
    if end > start:
        embs[:, start:end] = 0
    emb8 = np.clip(embs, -240, 240).astype(f8)
    q32 = np.ascontiguousarray(q, dtype=np.float32)
    q8 = np.clip(q32, -240, 240).astype(f8)
    # qt[t, i, p, m] = q8[m, t*256 + i*128 + p]
    qt = np.ascontiguousarray(q8.T.reshape(KT2, 2, 128, Q))
    in_maps = []
    for c in range(NCORES):
        lo = c * NSHARD
        pad = np.zeros((D, NPAD), dtype=f8)
        pad[:, :NSHARD] = emb8[:, lo:lo + NSHARD]
        # embL[j, p, (t*2+i)*CHUNK + n] = pad[t*256 + i*128 + p, j*CHUNK + n]
        embL = np.ascontiguousarray(
            pad.reshape(KT2 * 2, 128, NCH, CHUNK).transpose(2, 1, 0, 3)
        ).reshape(NCH, 128, KT2 * 2 * CHUNK)
        embT = np.ascontiguousarray(emb[:, lo:lo + NSHARD].T)
        in_maps.append({"q": q32, "qT": qt, "embL": embL, "embT": embT})
    return in_maps


def kernel(query, embeddings, start, end):
    q = np.asarray(query, dtype=np.float32)
    emb = np.asarray(embeddings, dtype=np.float32)
    start_i = int(np.asarray(start))
    end_i = int(np.asarray(end))
    assert q.shape == (Q, D) and emb.shape == (D, N)

    nc = _get_program()
    in_maps = _prepare_core_inputs(q, emb, start_i, end_i)

    trace = os.environ.get("KNN_TRACE", "0") == "1"
    if trace:
        _install_ntff_hook_shim()
    res = bass_utils.run_bass_kernel_spmd(
        nc, in_maps, core_ids=list(range(NCORES)), trace=trace)
    if trace:
        _prog_cache["last_exec_time_ns"] = res.exec_time_ns
        _prog_cache["last_results"] = res

    vals = np.stack([r["out_vals"] for r in res.results])          # [8, Q, 6]
    ids = np.stack([r["out_ids"] for r in res.results]).astype(np.int64)
    np.clip(ids, 0, NSHARD - 1, out=ids)
    gids = ids + (np.arange(NCORES, dtype=np.int64) * NSHARD)[:, None, None]

    allv = vals.transpose(1, 0, 2).reshape(Q, NCORES * L2K)
    allg = gids.transpose(1, 0, 2).reshape(Q, NCORES * L2K)
    # top-4 by value desc, index asc on ties (jax.lax.top_k tie rule)
    order = np.lexsort((allg, -allv), axis=1)[:, :TOPK]
    top_v = np.take_along_axis(allv, order, axis=1).astype(np.float32)
    top_i = np.take_along_axis(allg, order, axis=1).astype(np.int32)
    return top_v, top_i
